# revision 1
# baseline (speedup 1.0000x reference)
"""Trainium2 Bass kernel for nn_CombinedLoss (L1 wave + L1 on real-morlet CWT).

Math: loss = 0.5*mean|o-t| + 0.5*mean|CWT(o)-CWT(t)|.  Convolution is
linear, so CWT(o)-CWT(t) = CWT(o-t): one CWT pass over d = o-t.

Mapping: the 1-D convs (36 widths, taps 10..360) are banded-Toeplitz
matmuls on the tensor engine.  The signal is laid out transposed
(D_T[u, col] = d[128*col + u]) so the PE contracts over 128 consecutive
samples per chunk; each width needs 3 or 5 chunk matmuls (130 total).

Sharding: positions are split across the 8 cores (32768 samples each,
with 256-sample halos, zero-padded at the global edges) so every core
runs the identical SPMD program; per-core partial |.| sums are gathered
and combined on the host (the all-reduce step).
"""

import numpy as np
import ml_dtypes

import concourse.bass as bass
import concourse.tile as tile
import concourse.mybir as mybir
from concourse.bass_utils import run_bass_kernel_spmd
from concourse.masks import make_identity
from concourse.vector_clock import ScopedClock

L = 262144
NW = 36
ALPHA = 0.5
N_CORES = 8
CORE_POS = L // N_CORES          # 32768 positions per core
WIN = 49152                      # 3 chunks of 16384 (256-halo + pad)
NGROUPS = 9                      # 4 widths per reduce group
F32 = mybir.dt.float32
BF16 = mybir.dt.bfloat16
WDT = mybir.dt.bfloat16          # weight/signal dtype on the PE
WDT_NP = mybir.dt.np(WDT)


class _TC(tile.TileContext):
    """TileContext whose tail drain carries at most one sync wait.

    The walrus build in this container rejects a Drain instruction with
    more than one sync wait; emit the global-clock waits as standalone
    wait_ge instructions instead.
    """

    def _lower_ordered_insts(self, ordered):
        # Hoist all-but-one sync wait off each instruction into standalone
        # EventSemaphore waits on the same engine (in-order execution makes
        # this equivalent); walrus here allows 1 wait per instruction.
        nc = self.nc
        for bb_name in list(ordered.keys()):
            insts = ordered[bb_name]
            new = []
            for inst in insts:
                si = inst.sync_info
                if si is not None and len(si.on_wait) > 1:
                    waits = list(si.on_wait)
                    for w in waits[:-1]:
                        nop = mybir.InstEventSemaphore(
                            name=f"wsplit-{nc.next_id()}", ins=[], outs=[],
                            engine=inst.engine,
                        )
                        nop.sync_info = mybir.SyncInfo(on_wait=[w], on_update=[])
                        nc.register_instruction(nop, overwrite=True)
                        new.append(nop)
                    inst.sync_info = mybir.SyncInfo(
                        on_wait=[waits[-1]], on_update=list(si.on_update)
                    )
                new.append(inst)
            ordered[bb_name] = new
        return super()._lower_ordered_insts(ordered)

    def _drain_and_barrier(self, tick_clock, wait_clock):
        nc = self.nc
        probe = mybir.InstDrain(
            name=f"probe-{nc.next_id()}", ins=[], outs=[], engine=mybir.EngineType.SP
        )
        wait_clock.add_sem_waits(probe, ScopedClock({None: tick_clock.global_clock}))
        si = probe.sync_info
        waits = list(si.on_wait) if si is not None else []
        allocated = self.sems.allocated()
        handles = list(allocated.values()) if isinstance(allocated, dict) else list(allocated)
        id2sem = {h.num: h for h in handles}
        name2sem = {h.name: h for h in handles}
        for w in waits:
            sem = id2sem.get(w.id) or name2sem.get(w.ant_name)
            assert sem is not None, (w.id, w.ant_name, sorted(id2sem))
            nc.sync.wait_ge(sem, w.wait_value)
        nc.sync.drain()
        nc.all_engine_barrier()
        popped = nc._tile_sem_poison_stack.pop()
        assert popped is self._sem_poison
        nc.clear_and_free_semaphores(list(self.sems.allocated().values()))
        nc.all_engine_barrier()


def _morlet_flipped(N, w):
    # reference convolves with ker[::-1] of the real morlet; convolution
    # out[i] = sum_k g[k] d[i - a0 + k] uses g = that kernel re-flipped.
    x = np.linspace(-2.0 * np.pi, 2.0 * np.pi, N)
    ker = (np.cos(w * x) - np.exp(-0.5 * w * w)) * np.exp(-0.5 * x * x) * np.pi ** (-0.25)
    return ker  # ker[::-1][::-1]


def _width_meta(w):
    N = 10 * w
    a0 = 5 * w
    q = -(-a0 // 128)
    nch = (127 + (N - 1) - a0 + 128 * q) // 128 + 1
    return N, a0, q, nch


def _build_weights():
    """[128, 130*128] bf16 Toeplitz chunks, widths 1..36 in order, plus
    per-width (q, nch, tile_offset)."""
    mats = []
    meta = []
    off = 0
    for w in range(1, NW + 1):
        N, a0, q, nch = _width_meta(w)
        g = _morlet_flipped(N, float(w))
        up = np.arange(128)[:, None]
        j = np.arange(128)[None, :]
        for cc in range(nch):
            k = 128 * cc + up - j + a0 - 128 * q
            M = np.where((k >= 0) & (k < N), g[np.clip(k, 0, N - 1)], 0.0)
            mats.append(M)
        meta.append((q, nch, off))
        off += nch
    T = np.concatenate(mats, axis=1)  # [128, 130*128]
    return T.astype(WDT_NP), meta


_T_WEIGHTS, _W_META = _build_weights()
_N_TILES = _T_WEIGHTS.shape[1] // 128  # 130

# group g covers widths 4g+1 .. 4g+4
_GROUPS = []
for g in range(NGROUPS):
    ws = list(range(4 * g + 1, 4 * g + 5))
    ch0 = _W_META[ws[0] - 1][2]
    nch_g = sum(_W_META[w - 1][1] for w in ws)
    _GROUPS.append((ws, ch0, nch_g))

_NC_CACHE = None


def _build_nc():
    nc = bass.Bass("TRN2", target_bir_lowering=False, debug=False, num_devices=N_CORES)
    o_ext = nc.dram_tensor("o_win", [128, 384], F32, kind="ExternalInput")
    t_ext = nc.dram_tensor("t_win", [128, 384], F32, kind="ExternalInput")
    tw_ext = nc.dram_tensor("tw", [128, _N_TILES * 128], WDT, kind="ExternalInput")
    out_ext = nc.dram_tensor("partials", [128, 16], F32, kind="ExternalOutput")

    with _TC(nc) as tc:
        with (
            tc.tile_pool(name="const", bufs=1) as const_pool,
            tc.tile_pool(name="sig", bufs=1) as sig_pool,
            tc.tile_pool(name="dnat", bufs=1) as dnat_pool,
            tc.tile_pool(name="dt", bufs=1) as dt_pool,
            tc.tile_pool(name="wslab", bufs=1) as wslab_pool,
            tc.tile_pool(name="scratch", bufs=2) as scratch_pool,
            tc.tile_pool(name="parts", bufs=1) as parts_pool,
            tc.tile_pool(name="psd", bufs=1, space="PSUM") as psd_pool,
            tc.tile_pool(name="psc", bufs=3, space="PSUM") as psc_pool,
        ):
            ident = const_pool.tile([128, 128], BF16, tag="ident")
            make_identity(nc, ident[:])

            # PE warm-up: ~3us of dummy transposes (no data deps) so the
            # p-state/HAM ramp runs while the input DMAs are in flight.
            warm_ps = psd_pool.tile([128, 128], BF16, tag="warm")
            for _ in range(26):
                nc.tensor.transpose(warm_ps[:], ident[:], ident[:])

            # weight slabs: 3 big DMAs (3 reduce-groups each) for
            # descriptor efficiency while still overlapping with PE
            slab_tiles = []
            slab_of_group = {}
            col_in_slab = {}
            for s in range(3):
                gs = _GROUPS[3 * s:3 * s + 3]
                ch0 = gs[0][1]
                nch_s = sum(g[2] for g in gs)
                t = wslab_pool.tile([128, nch_s * 128], WDT, tag=f"w{s}")
                nc.sync.dma_start(t[:], tw_ext[:, ch0 * 128:(ch0 + nch_s) * 128])
                slab_tiles.append(t)
                for gi, (ws_, gch0, gnch) in enumerate(gs):
                    slab_of_group[3 * s + gi] = t
                    col_in_slab[3 * s + gi] = gch0 - ch0

            o_sb = sig_pool.tile([128, 384], F32, tag="o")
            nc.sync.dma_start(o_sb[:], o_ext[:])
            t_sb = sig_pool.tile([128, 384], F32, tag="t")
            nc.sync.dma_start(t_sb[:], t_ext[:])

            d_nat = dnat_pool.tile([128, 384], BF16)
            nc.vector.tensor_sub(d_nat[:], o_sb[:], t_sb[:])

            psum_d = psd_pool.tile([128, 384], BF16)
            for c in range(3):
                nc.tensor.transpose(
                    psum_d[:, 128 * c:128 * (c + 1)],
                    d_nat[:, 128 * c:128 * (c + 1)],
                    ident[:],
                )
            dt = dt_pool.tile([128, 384], WDT)
            nc.vector.tensor_copy(dt[:], psum_d[:])

            parts = parts_pool.tile([128, 16], F32)
            # wave term: own positions are D_T columns 2..258 (bf16 psum)
            nc.vector.tensor_reduce(
                parts[:, 0:1], psum_d[:, 2:258], axis=mybir.AxisListType.X,
                op=mybir.AluOpType.add, apply_absolute_value=True,
            )

            for g, (ws, ch0, nch_g) in enumerate(_GROUPS):
                psum = psc_pool.tile([128, 1024], F32, tag="conv")
                wsl = slab_of_group[g]
                toff = col_in_slab[g]
                for k, w in enumerate(ws):
                    q, nch, _ = _W_META[w - 1]
                    for cc in range(nch):
                        c0 = 2 - q + cc
                        nc.tensor.matmul(
                            psum[:, 256 * k:256 * (k + 1)],
                            wsl[:, 128 * toff:128 * (toff + 1)],
                            dt[:, c0:c0 + 256],
                            start=(cc == 0),
                            stop=(cc == nch - 1),
                        )
                        toff += 1
                if g % 2 == 0:
                    nc.vector.tensor_reduce(
                        parts[:, 1 + g:2 + g], psum[:], axis=mybir.AxisListType.X,
                        op=mybir.AluOpType.add, apply_absolute_value=True,
                    )
                else:
                    sc = scratch_pool.tile([128, 1024], F32, tag="absout")
                    nc.scalar.activation(
                        sc[:], psum[:], mybir.ActivationFunctionType.Abs,
                        accum_out=parts[:, 1 + g:2 + g],
                    )

            nc.gpsimd.dma_start(out_ext[:], parts[:])
    return nc


def _get_nc():
    global _NC_CACHE
    if _NC_CACHE is None:
        _NC_CACHE = _build_nc()
    return _NC_CACHE


def kernel(outputs, targets):
    o = np.asarray(outputs, dtype=np.float32).reshape(-1)
    t = np.asarray(targets, dtype=np.float32).reshape(-1)
    assert o.shape == (L,) and t.shape == (L,)

    in_maps = []
    for core in range(N_CORES):
        win_start = core * CORE_POS - 256
        lo, hi = max(0, win_start), min(L, win_start + WIN)
        o_win = np.zeros(WIN, np.float32)
        t_win = np.zeros(WIN, np.float32)
        o_win[lo - win_start:hi - win_start] = o[lo:hi]
        t_win[lo - win_start:hi - win_start] = t[lo:hi]
        # tile[p, 128c+q] = win[c*16384 + 128p + q]
        o_tile = o_win.reshape(3, 128, 128).transpose(1, 0, 2).reshape(128, 384)
        t_tile = t_win.reshape(3, 128, 128).transpose(1, 0, 2).reshape(128, 384)
        in_maps.append({
            "o_win": np.ascontiguousarray(o_tile),
            "t_win": np.ascontiguousarray(t_tile),
            "tw": _T_WEIGHTS,
        })

    nc = _get_nc()
    res = run_bass_kernel_spmd(nc, in_maps, core_ids=list(range(N_CORES)))

    wave = 0.0
    cwt = 0.0
    for core in range(N_CORES):
        p = np.asarray(res.results[core]["partials"], dtype=np.float64)
        wave += p[:, 0].sum()
        cwt += p[:, 1:1 + NGROUPS].sum()
    loss = ALPHA * wave / L + (1.0 - ALPHA) * cwt / (NW * L)
    return np.float32(loss)

